# revision 18
# baseline (speedup 1.0000x reference)
"""Causal self-attention (B=4, T=2048, C=1024, H=16) on 8 TRN2 NeuronCores.

Sharding: tensor-parallel over heads. Each core owns 2 heads:
  - computes its 384-column slice of the QKV projection (q|k|v, 128 cols each)
    directly in transposed layout qkvT = w_slice.T @ xT (x is host-pre-transposed),
  - runs causal attention for its 8 (batch, head) pairs in scores-transposed
    form sT = kT.T @ qT so that softmax(p) feeds the p@v matmul with no PE
    transposes (pT == sT); the softmax normalizer Z is accumulated by an
    appended ones-column in the v stationary operand,
  - normalizes y by 1/Z and multiplies by its 128-row slice of w_proj,
    producing a partial [8192, 1024] output.
Host sums the 8 partials and adds b_proj.

All matmuls run in float32r (fp32 fast path, 1 cycle/row at N>=256).
"""

import numpy as np

import concourse.bass as bass
import concourse.mybir as mybir
import concourse.tile as tile
from concourse import bacc
from concourse.bass_utils import run_bass_kernel_spmd
from concourse.masks import make_identity

B, T, C, H, D = 4, 2048, 1024, 16, 64
NCORES = 8
HPC = H // NCORES          # heads per core = 2
SH = HPC * D               # 128: shard width of each of q/k/v
R = B * T                  # 8192 rows
KC = C // 128              # 8 contraction chunks
NT = R // 512              # 16 row-groups of 512
QG = T // 512              # 4 query groups per (batch, head) pair
CPB = T // 128             # 16 key chunks per batch
f32 = mybir.dt.float32
f32r = mybir.dt.float32r
EXP = mybir.ActivationFunctionType.Exp

TRACE = False
TRACE_KWARGS = {}
LAST_RESULT = None
_NC_CACHE = None


def _emit(tc, xT, w_s, b_s, wp_s, out):
    nc = tc.nc
    # DRAM bounce for transposing softmax-denominator rows into column layout
    zdram = nc.dram_tensor("zdram", [2 * B * QG * 512], f32, kind="Internal").ap()

    with (
        tc.tile_pool(name="const", bufs=1) as constp,
        tc.tile_pool(name="persist", bufs=1) as persist,
        tc.tile_pool(name="xt", bufs=10) as xtp,
        tc.tile_pool(name="vstage", bufs=2) as vstagep,
        tc.tile_pool(name="pt", bufs=3) as ptp,
        tc.tile_pool(name="yu", bufs=2) as yup,
        tc.tile_pool(name="zcol", bufs=2) as zcolp,
        tc.tile_pool(name="ztmp", bufs=2) as ztmpp,
        tc.tile_pool(name="ostage", bufs=4) as ostagep,
    ):
        ident = constp.tile([128, 128], f32)
        make_identity(nc, ident[:])
        ones_f = constp.tile([128, 128], f32)
        nc.gpsimd.memset(ones_f[:], 1.0)
        w_sb = constp.tile([128, KC, 3 * SH], f32r)
        nc.sync.dma_start(
            w_sb[:], w_s.rearrange("(kc p) m -> p kc m", p=128).bitcast(f32r)
        )
        b_sb = constp.tile([128, 3], f32)
        nc.sync.dma_start(b_sb[:], b_s.rearrange("(m p) -> p m", p=128))
        wp_sb = constp.tile([128, C], f32r)
        nc.sync.dma_start(wp_sb[:], wp_s.bitcast(f32r))

        qT = persist.tile([128, R], f32r)
        kT = persist.tile([128, R], f32r)
        # v per (pair, key-chunk): [128, 65] blocks; col 64 = 1.0 (Z accumulator)
        vaug = persist.tile([128, 2 * B * CPB * 65], f32r)
        nc.vector.tensor_copy(
            vaug[:].rearrange("p (blk c) -> p blk c", c=65)[:, :, 64:65],
            ones_f[:].unsqueeze(2),
        )

        # ---- Phase 1: qkvT = w_s.T @ xT, plus v transposed into vaug ----
        with (
            tc.tile_pool(name="ps_qkv", bufs=3, space="PSUM") as ps_qkv,
            tc.tile_pool(name="ps_t", bufs=2, space="PSUM") as ps_t,
        ):
            for n in range(NT):
                xts = []
                for k in range(KC):
                    xt = xtp.tile([128, 512], f32r, name=f"xt_{n}_{k}", tag="xt")
                    nc.sync.dma_start(
                        xt[:],
                        xT[128 * k : 128 * (k + 1), 512 * n : 512 * (n + 1)].bitcast(
                            f32r
                        ),
                    )
                    xts.append(xt)
                for m in range(3):
                    ps = ps_qkv.tile([128, 512], f32, name=f"ps_{n}_{m}", tag="ps")
                    for k in range(KC):
                        nc.tensor.matmul(
                            ps[:],
                            w_sb[:, k, 128 * m : 128 * (m + 1)],
                            xts[k][:],
                            start=(k == 0),
                            stop=(k == KC - 1),
                        )
                    nsl = slice(512 * n, 512 * (n + 1))
                    if m == 0:
                        nc.vector.tensor_scalar_add(qT[:, nsl], ps[:], b_sb[:, 0:1])
                    elif m == 1:
                        nc.vector.tensor_scalar_add(kT[:, nsl], ps[:], b_sb[:, 1:2])
                    else:
                        vst = vstagep.tile([128, 512], f32, name=f"vst_{n}", tag="vst")
                        nc.vector.tensor_scalar_add(vst[:], ps[:], b_sb[:, 2:3])
                        b_idx = n // 4
                        for j in range(4):
                            c_local = (n % 4) * 4 + j
                            tp = ps_t.tile([128, 128], f32, name=f"tp_{n}_{j}", tag="tp")
                            nc.tensor.transpose(
                                tp[:], vst[:, 128 * j : 128 * (j + 1)], ident[:]
                            )
                            for h in range(2):
                                off = ((b_idx * 2 + h) * CPB + c_local) * 65
                                nc.vector.tensor_copy(
                                    vaug[:, off : off + 64], tp[:, 64 * h : 64 * h + 64]
                                )

        # ---- Phase 2: per-batch attention + partial projection ----
        with (
            tc.tile_pool(name="ps_s", bufs=2, space="PSUM") as ps_s,
            tc.tile_pool(name="ps_o", bufs=1, space="PSUM") as ps_o,
            tc.tile_pool(name="ps_p", bufs=3, space="PSUM") as ps_p,
        ):
            for b in range(B):
                yu = yup.tile([128, T], f32r, name=f"yu_{b}", tag="yu")
                # zcol[p, h*16 + rt] = softmax denom Z for (head h, t = 128*rt + p)
                zcol = zcolp.tile([128, 32], f32, name=f"zcol_{b}", tag="zcol")
                for h in range(2):
                    pair = b * 2 + h
                    hsl = slice(64 * h, 64 * h + 64)
                    for g in range(QG):
                        ot = ps_o.tile([65, 512], f32, name=f"ot_{pair}_{g}", tag="ot")
                        nkc = 4 * g + 4
                        for kc2 in range(nkc // 2):
                            sp = ps_s.tile(
                                [128, 1024], f32, name=f"sp_{pair}_{g}_{kc2}", tag="sp"
                            )
                            for half in range(2):
                                kc = 2 * kc2 + half
                                nc.tensor.matmul(
                                    sp[:, 512 * half : 512 * (half + 1)],
                                    kT[hsl, b * T + 128 * kc : b * T + 128 * (kc + 1)],
                                    qT[hsl, b * T + 512 * g : b * T + 512 * (g + 1)],
                                    start=True,
                                    stop=True,
                                )
                            pt = ptp.tile(
                                [128, 1024], f32r, name=f"pt_{pair}_{g}_{kc2}", tag="pt"
                            )
                            nc.scalar.activation(pt[:], sp[:], EXP, scale=0.125)
                            for half in range(2):
                                kc = 2 * kc2 + half
                                j = kc - 4 * g
                                if j >= 0:
                                    # keep iff f >= 128*j + p  (tk <= tq)
                                    nc.gpsimd.affine_select(
                                        out=pt[:, 512 * half : 512 * (half + 1)],
                                        in_=pt[:, 512 * half : 512 * (half + 1)],
                                        compare_op=mybir.AluOpType.is_ge,
                                        fill=0.0,
                                        base=-(128 * j),
                                        channel_multiplier=-1,
                                        pattern=[[1, 512]],
                                    )
                            for half in range(2):
                                kc = 2 * kc2 + half
                                off = (pair * CPB + kc) * 65
                                nc.tensor.matmul(
                                    ot[:],
                                    vaug[:, off : off + 65],
                                    pt[:, 512 * half : 512 * (half + 1)],
                                    start=(kc == 0),
                                    stop=(kc == nkc - 1),
                                )
                        gsl = slice(512 * g, 512 * (g + 1))
                        stage = ztmpp.tile(
                            [65, 512], f32r, name=f"stage_{pair}_{g}", tag="stage"
                        )
                        nc.vector.tensor_copy(stage[:], ot[:])
                        # y rows -> yu at this head's partition block (DMA may shift
                        # partitions; engines may not)
                        nc.sync.dma_start(yu[hsl, gsl], stage[0:64, :])
                        # transpose Z row [1, 512] -> zcol [128, 4] (t = 128*tt + p)
                        zd = zdram[(pair * QG + g) * 512 : (pair * QG + g + 1) * 512]
                        nc.sync.dma_start(zd, stage[64:65, :].bitcast(f32))
                        nc.sync.dma_start(
                            zcol[:, 16 * h + 4 * g : 16 * h + 4 * (g + 1)],
                            zd.rearrange("(tt p) -> p tt", p=128),
                        )
                    zsl = slice(16 * h, 16 * (h + 1))
                    nc.vector.reciprocal(zcol[:, zsl], zcol[:, zsl])
                # partial projection for batch b
                for rt in range(T // 128):
                    rsl = slice(128 * rt, 128 * (rt + 1))
                    r0 = b * T + 128 * rt
                    for jn in range(2):
                        nsl = slice(512 * jn, 512 * (jn + 1))
                        pp0 = ps_p.tile(
                            [128, 512], f32, name=f"pp0_{b}_{rt}_{jn}", tag="pp"
                        )
                        pp1 = ps_p.tile(
                            [128, 512], f32, name=f"pp1_{b}_{rt}_{jn}", tag="pp"
                        )
                        nc.tensor.matmul(
                            pp0[:], yu[0:64, rsl], wp_sb[0:64, nsl],
                            start=True, stop=True,
                        )
                        nc.tensor.matmul(
                            pp1[:], yu[64:128, rsl], wp_sb[64:128, nsl],
                            start=True, stop=True,
                        )
                        ost = ostagep.tile(
                            [128, 512], f32, name=f"ost_{b}_{rt}_{jn}", tag="ost"
                        )
                        nc.scalar.activation(
                            ost[:],
                            pp0[:],
                            mybir.ActivationFunctionType.Copy,
                            scale=zcol[:, rt : rt + 1],
                        )
                        nc.vector.scalar_tensor_tensor(
                            ost[:],
                            pp1[:],
                            zcol[:, 16 + rt : 16 + rt + 1],
                            ost[:],
                            op0=mybir.AluOpType.mult,
                            op1=mybir.AluOpType.add,
                        )
                        nc.sync.dma_start(out[r0 : r0 + 128, nsl], ost[:])


def build_nc():
    global _NC_CACHE
    if _NC_CACHE is not None:
        return _NC_CACHE
    nc = bacc.Bacc("TRN2", target_bir_lowering=False, debug=False)
    xT = nc.dram_tensor("xT", [C, R], f32, kind="ExternalInput").ap()
    w_s = nc.dram_tensor("w_s", [C, 3 * SH], f32, kind="ExternalInput").ap()
    b_s = nc.dram_tensor("b_s", [3 * SH], f32, kind="ExternalInput").ap()
    wp_s = nc.dram_tensor("wp_s", [SH, C], f32, kind="ExternalInput").ap()
    out = nc.dram_tensor("out", [R, C], f32, kind="ExternalOutput").ap()
    with tile.TileContext(nc) as tc:
        _emit(tc, xT, w_s, b_s, wp_s, out)
    nc.compile()
    _NC_CACHE = nc
    return nc


def kernel(x, w_attn, b_attn, w_proj, b_proj):
    global LAST_RESULT
    x = np.asarray(x, dtype=np.float32)
    w_attn = np.asarray(w_attn, dtype=np.float32)
    b_attn = np.asarray(b_attn, dtype=np.float32)
    w_proj = np.asarray(w_proj, dtype=np.float32)
    b_proj = np.asarray(b_proj, dtype=np.float32)

    xTh = np.ascontiguousarray(x.reshape(R, C).T)  # [C, R]
    in_maps = []
    for c in range(NCORES):
        csl = slice(SH * c, SH * (c + 1))
        w_s = np.ascontiguousarray(
            np.concatenate(
                [w_attn[:, csl], w_attn[:, C:][:, csl], w_attn[:, 2 * C :][:, csl]],
                axis=1,
            )
        )
        b_s = np.ascontiguousarray(
            np.concatenate([b_attn[csl], b_attn[C:][csl], b_attn[2 * C :][csl]])
        )
        wp_s = np.ascontiguousarray(w_proj[csl, :])
        in_maps.append({"xT": xTh, "w_s": w_s, "b_s": b_s, "wp_s": wp_s})

    nc = build_nc()
    res = run_bass_kernel_spmd(
        nc,
        in_maps,
        core_ids=list(range(NCORES)),
        trace=TRACE,
        **TRACE_KWARGS,
    )
    LAST_RESULT = res
    acc = np.zeros((R, C), dtype=np.float64)
    for c in range(NCORES):
        acc += res.results[c]["out"]
    out = (acc + b_proj.astype(np.float64)).astype(np.float32)
    return out.reshape(B, T, C)
